# revision 19
# baseline (speedup 1.0000x reference)
"""Multi-head attention (RoPE, causal) on 8 TRN2 NeuronCores.

Sharding: core c -> batch b = c//2, head-group g = c%2 (8 of 16 heads).

v5: bf16 matmul operands (FWL weight loads, 2x DVE mode, half DMA),
row-packed score matmuls (two dk=64 heads concurrently in the PE array
on partition halves) into one 2-bank PSUM tile so exp/tri-mask run as
single strided ACT/DVE instructions per k-tile; per-chunk SBUF pair
assembly so attention never waits on a bulk staging step; PV pipelined
one k-tile behind scores; p1 projections and per-chunk o_proj
interleaved into the attention stream to fill ACT-bound bubbles and
keep HAM warm.

Head pairs (local): (0,1),(2,3) from p-group 0, (4,5),(6,7) from
p-group 1. Pair pr holds head 2pr on partitions 0-63 and head 2pr+1 on
partitions 64-127 of its q/k tiles.

PSUM banks: pab 2 + pSAB 4 + pO_A 1 + pO_B 1 = 8.
"""

import numpy as np
from contextlib import ExitStack

from ml_dtypes import bfloat16

import concourse.bacc as bacc
import concourse.bass as bass
import concourse.mybir as mybir
import concourse.tile as tile
from concourse.bass_utils import run_bass_kernel_spmd
from concourse.masks import make_upper_triangular

F32 = mybir.dt.float32
BF16 = mybir.dt.bfloat16
AF = mybir.ActivationFunctionType

D = 1024
S = 2048
NH = 16
DK = 64
HPC = 8          # heads per core
HD = HPC * DK    # 512
NCORES = 8
THETA = 10000.0

NS = S // 128    # 16 s-tiles (k tiles in attention)
NC_ = 4          # s-chunks of 512
NK = D // 128    # 8 d-tiles of x

_CACHE = {}


def _copy(nc, use_scalar, out, in_):
    if use_scalar:
        nc.scalar.copy(out, in_)
    else:
        nc.vector.tensor_copy(out, in_)


def _emit_v_group(nc, psp, wvall, xtall, vpall, ones8, jlo, jhi):
    """V projection for s-tiles [jlo, jhi): vpall[:, j, h*65:(h+1)*65] =
    [v (64 cols), ones (1 col)] per head."""
    for j in range(jlo, jhi):
        psv = psp.tile([128, 512], F32, tag="pab", bufs=2, name="psv")
        jc, jr = j // 4, j % 4
        for k in range(NK):
            nc.tensor.matmul(
                psv, xtall[:, jc, k, jr * 128:(jr + 1) * 128],
                wvall[:, k, :],
                start=(k == 0), stop=(k == NK - 1))
        vslice = vpall[:, j, :].rearrange("p (h e) -> p h e", e=65)
        nc.scalar.copy(vslice[:, :, 0:64],
                       psv.rearrange("p (h e) -> p h e", e=64))
        nc.vector.tensor_copy(vslice[:, :, 64], ones8)


def _qk_proj_pieces(nc, psp, tp, o12p, wsb, c, xtall, cos_sb,
                    sin_sb, pair_a, pair_b, p):
    """One s-chunk of a q/k projection + rope, as a list of small
    closures, each emitting ~1 PE matmul (plus tail DVE/DMA work), so
    the caller can splice them between attention k-tiles to fill
    ACT-bound PE bubbles.  Copies run on DVE so the feed never delays
    the exp stream on ACT.
    """
    cs = slice(c * 512, (c + 1) * 512)
    st = {}

    def tail1():
        s1b = tp.tile([128, 512], BF16, tag="s1b", name="s1b")
        nc.vector.tensor_copy(s1b, st['ps1'])
        st['s1b'] = s1b

    def tail2():
        s2b = tp.tile([128, 512], BF16, tag="s2b", name="s2b")
        nc.vector.tensor_copy(s2b, st['ps2'])
        s1b = st['s1b']
        cosc = cos_sb[:, cs]
        sinc = sin_sb[:, cs]
        o12 = o12p.tile([128, 2, 512], BF16, tag="o12", bufs=3,
                        name="o12")
        tA = tp.tile([128, 512], BF16, tag="rt", bufs=4, name="tA")
        nc.vector.tensor_mul(tA, s1b, cosc)
        tB = tp.tile([128, 512], BF16, tag="rt", bufs=4, name="tB")
        nc.vector.tensor_mul(tB, s2b, sinc)
        nc.vector.tensor_sub(o12[:, 0, :], tA, tB)
        tC = tp.tile([128, 512], BF16, tag="rt", bufs=4, name="tC")
        nc.vector.tensor_mul(tC, s1b, sinc)
        tD = tp.tile([128, 512], BF16, tag="rt", bufs=4, name="tD")
        nc.vector.tensor_mul(tD, s2b, cosc)
        nc.vector.tensor_add(o12[:, 1, :], tC, tD)
        # scatter into pair tiles: local head i -> pair (2p + i//2),
        # partition half i%2; x1 rows +0, x2 rows +32
        engs = [nc.sync, nc.gpsimd]
        for i in range(4):
            dst = pair_a if i < 2 else pair_b
            hp = i % 2
            for x in range(2):
                eng = engs[(2 * i + x) % 2]
                eng.dma_start(
                    out=dst[64 * hp + 32 * x:64 * hp + 32 * x + 32, cs],
                    in_=o12[32 * i:32 * i + 32, x, :])

    def mk(i):
        def go():
            if i == 0:
                st['ps1'] = psp.tile([128, 512], F32, tag="pab",
                                     bufs=2, name="ps1")
            if i == NK:
                st['ps2'] = psp.tile([128, 512], F32, tag="pab",
                                     bufs=2, name="ps2")
            if i < NK:
                nc.tensor.matmul(st['ps1'], wsb[:, i, 0:128],
                                 xtall[:, c, i, :],
                                 start=(i == 0), stop=(i == NK - 1))
                if i == NK - 1:
                    tail1()
            else:
                k = i - NK
                nc.tensor.matmul(st['ps2'], wsb[:, k, 128:256],
                                 xtall[:, c, k, :],
                                 start=(k == 0), stop=(k == NK - 1))
                if k == NK - 1:
                    tail2()
        return go

    return [mk(i) for i in range(2 * NK)]


class _Filler:
    """FIFO of PE-work closures spliced into attention k-tile loops."""

    def __init__(self):
        self.pieces = []
        self.rate = 2

    def add(self, pieces):
        self.pieces.extend(pieces)

    def feed(self, n):
        for _ in range(min(n, len(self.pieces))):
            self.pieces.pop(0)()

    def drain(self):
        self.feed(len(self.pieces))


def _emit_pair_chunk(nc, psp, ptp, rp, pr, Q, qpair, kpair, vpall, yin,
                     tri2, filler):
    """Causal attention for head pair pr on q-chunk Q (cols 512Q..512Q+512).

    Scores for the two heads are row-packed (head A on PE rows 0-63,
    head B on rows 64-127, concurrent) into one [128, 1024] PSUM tile;
    exp and the diagonal mask then run as single strided instructions
    over both heads.  PV is emitted one k-tile behind scores so the PE
    never waits on ACT latency.  Writes normalized outputs into
    yin[0:64] (head 2pr) and yin[64:128] (head 2pr+1, via SBUF DMA).
    """
    jmax = 4 * Q + 3
    q0 = 512 * Q
    pO = [psp.tile([128, 512], F32, tag=f"pO{s}", bufs=1, name=f"pO{s}")
          for s in range(2)]

    def exp_pv(j, pSAB):
        n0 = max(128 * j - q0, 0)
        pt = ptp.tile([128, 1024], BF16, tag="pt", bufs=3, name="pt")
        ptv = pt.rearrange("p (s c) -> p s c", s=2)
        psv = pSAB.rearrange("p (s c) -> p s c", s=2)
        nc.scalar.activation(ptv[:, :, n0:512], psv[:, :, n0:512],
                             AF.Exp, scale=0.125)
        if j >= 4 * Q:
            t2 = tri2.rearrange("p (s c) -> p s c", s=2)
            nc.vector.tensor_mul(ptv[:, :, n0:n0 + 128],
                                 ptv[:, :, n0:n0 + 128], t2)
        for s in range(2):
            h = 2 * pr + s
            vsl = vpall[:, j, h * 65:(h + 1) * 65]
            nc.tensor.matmul(pO[s][0:65, n0:512],
                             vsl, pt[:, 512 * s + n0:512 * s + 512],
                             start=(j == 0), stop=(j == jmax))

    inflight = []
    for j in range(jmax + 1):
        off = 128 * j
        n0 = max(off - q0, 0)          # in-chunk col where k<=q begins
        pSAB = psp.tile([128, 1024], F32, tag="pS", bufs=2, name="pSAB")
        for s in range(2):
            bp = 64 * s
            nc.tensor.matmul(
                pSAB[:, 512 * s + n0:512 * s + 512],
                kpair[bp:bp + 64, off:off + 128],
                qpair[bp:bp + 64, q0 + n0:q0 + 512],
                start=True, stop=True)
        inflight.append((j, pSAB))
        filler.feed(filler.rate)
        if len(inflight) >= 2:
            exp_pv(*inflight.pop(0))
    for item in inflight:
        filler.feed(filler.rate)
        exp_pv(*item)

    for s in range(2):
        oc = rp.tile([64, 512], BF16, tag="oc", name="oc")
        nc.vector.tensor_copy(oc, pO[s][0:64, :])
        den = rp.tile([1, 512], F32, tag="den", name="den")
        nc.vector.tensor_copy(den, pO[s][64:65, :])
        rec = rp.tile([1, 512], F32, tag="rec", name="rec")
        nc.vector.reciprocal_approx_fast(rec, den)
        recb = rp.tile([64, 512], F32, tag="recb", name="recb")
        nc.gpsimd.partition_broadcast(recb, rec)
        if s == 0:
            nc.vector.tensor_mul(yin[0:64, :], oc, recb)
        else:
            ys = rp.tile([64, 512], BF16, tag="ys", name="ys")
            nc.vector.tensor_mul(ys, oc, recb)
            nc.sync.dma_start(out=yin[64:128, :], in_=ys)


def _oproj_pieces(nc, psp, ost, oct_sb, yins, OT, c):
    """Output projection for s-chunk c as spliceable closures (one PE
    matmul each): OT[:, cs] = sum_pr OC_pr.T @ yin_pr."""
    cs = slice(c * 512, (c + 1) * 512)
    st = {}

    def mk(dt, pr):
        def go():
            if pr == 0:
                st['pd'] = psp.tile([128, 512], F32, tag="pab", bufs=2,
                                    name="pd")
            nc.tensor.matmul(
                st['pd'], oct_sb[:, pr, dt * 128:(dt + 1) * 128],
                yins[pr], start=(pr == 0), stop=(pr == 3))
            if pr == 3:
                o_s = ost.tile([128, 512], BF16, tag="os", name="os")
                nc.vector.tensor_copy(o_s, st['pd'])
                eng = nc.gpsimd if dt % 2 == 0 else nc.sync
                eng.dma_start(out=OT[dt * 128:(dt + 1) * 128, cs],
                              in_=o_s)
        return go

    return [mk(dt, pr) for dt in range(8) for pr in range(4)]


def _build_nc():
    nc = bacc.Bacc(None, target_bir_lowering=False)

    # all inputs host-pre-arranged so every DMA is one contiguous
    # descriptor per partition
    XT = nc.dram_tensor("XT", [128, NC_, NK, 512], BF16,
                        kind="ExternalInput")
    WQ = nc.dram_tensor("WQ", [2, 128, NK, 256], BF16,
                        kind="ExternalInput")
    WK = nc.dram_tensor("WK", [2, 128, NK, 256], BF16,
                        kind="ExternalInput")
    WV = nc.dram_tensor("WV", [128, NK, HD], BF16, kind="ExternalInput")
    OC = nc.dram_tensor("OC", [128, 4, D], BF16, kind="ExternalInput")
    COS = nc.dram_tensor("COS", [128, S], BF16, kind="ExternalInput")
    SIN = nc.dram_tensor("SIN", [128, S], BF16, kind="ExternalInput")
    OT = nc.dram_tensor("OT", [D, S], BF16, kind="ExternalOutput")

    with tile.TileContext(nc) as tc, ExitStack() as ctx:
        const = ctx.enter_context(tc.tile_pool(name="const", bufs=1))
        resv = ctx.enter_context(tc.tile_pool(name="resv", bufs=1))
        psp = ctx.enter_context(tc.tile_pool(name="psp", bufs=2,
                                             space="PSUM"))

        # constants / inputs needed first, on distinct queues
        cos_sb = const.tile([128, S], BF16, tag="cos")
        nc.scalar.dma_start(out=cos_sb, in_=COS[:, :])
        sin_sb = const.tile([128, S], BF16, tag="sin")
        nc.scalar.dma_start(out=sin_sb, in_=SIN[:, :])

        xtall = resv.tile([128, NC_, NK, 512], BF16, tag="xtall")
        nc.sync.dma_start(out=xtall[:, 0], in_=XT[:, 0])
        wvall = resv.tile([128, NK, HD], BF16, tag="wvall")
        nc.sync.dma_start(out=wvall, in_=WV[:, :, :])

        tri2 = const.tile([128, 256], BF16, tag="tri2")
        make_upper_triangular(nc, tri2[:, 0:128], val=1.0, diag=True)
        make_upper_triangular(nc, tri2[:, 128:256], val=1.0, diag=True)
        vpall = resv.tile([128, NS, HPC * 65], BF16, tag="vpall")
        ones8 = const.tile([128, 8], BF16, tag="ones8")
        nc.vector.memset(ones8, 1.0)
        qpairs = [resv.tile([128, S], BF16, tag=f"qp{pr}", name=f"qp{pr}")
                  for pr in range(4)]
        kpairs = [resv.tile([128, S], BF16, tag=f"kp{pr}", name=f"kp{pr}")
                  for pr in range(4)]
        yin = [[resv.tile([128, 512], BF16, tag=f"yin{pr}_{Q}",
                          name=f"yin{pr}_{Q}")
                for Q in range(NC_)] for pr in range(4)]

        with tc.tile_pool(name="wp", bufs=1) as wp, \
             tc.tile_pool(name="tp", bufs=2) as tp, \
             tc.tile_pool(name="o12p", bufs=2) as o12p, \
             tc.tile_pool(name="ptp", bufs=2) as ptp, \
             tc.tile_pool(name="rp", bufs=2) as rp, \
             tc.tile_pool(name="ost", bufs=2) as ost:

            # p0 weights first (needed immediately), p1 + OC later on
            # the gpsimd queue
            wqk = {}
            for tname, Wt, eng in (("q", WQ, nc.sync), ("k", WK, nc.scalar)):
                w = wp.tile([128, NK, 256], BF16, tag=f"w{tname}0",
                            name=f"w{tname}0")
                eng.dma_start(out=w, in_=Wt[0])
                wqk[(tname, 0)] = w

            # remaining x chunks
            for c in range(1, NC_):
                eng = nc.sync if c % 2 == 0 else nc.scalar
                eng.dma_start(out=xtall[:, c], in_=XT[:, c])

            # startup: V projection and p0 q/k projections interleaved
            # per x-chunk so the PE starts as soon as chunk 0 lands
            fil = _Filler()
            for c in range(NC_):
                _emit_v_group(nc, psp, wvall, xtall, vpall, ones8,
                              4 * c, 4 * c + 4)
                fil.add(_qk_proj_pieces(nc, psp, tp, o12p, wqk[("q", 0)],
                                        c, xtall, cos_sb, sin_sb,
                                        qpairs[0], qpairs[1], 0))
                fil.drain()
                fil.add(_qk_proj_pieces(nc, psp, tp, o12p, wqk[("k", 0)],
                                        c, xtall, cos_sb, sin_sb,
                                        kpairs[0], kpairs[1], 0))
                fil.drain()

            # p1 weights + OC, now that the startup-critical loads are
            # through the queues
            for tname, Wt in (("q", WQ), ("k", WK)):
                w = wp.tile([128, NK, 256], BF16, tag=f"w{tname}1",
                            name=f"w{tname}1")
                nc.gpsimd.dma_start(out=w, in_=Wt[1])
                wqk[(tname, 1)] = w
            oct_sb = resv.tile([128, 4, D], BF16, tag="oct")
            nc.gpsimd.dma_start(out=oct_sb, in_=OC[:, :, :])

            # pair 0 attention, p1 q-projection spliced between k-tiles
            for Q in range(NC_):
                fil.add(_qk_proj_pieces(nc, psp, tp, o12p, wqk[("q", 1)],
                                        Q, xtall, cos_sb, sin_sb,
                                        qpairs[2], qpairs[3], 1))
                _emit_pair_chunk(nc, psp, ptp, rp, 0, Q, qpairs[0],
                                 kpairs[0], vpall, yin[0][Q], tri2, fil)
            fil.drain()
            # pair 1 attention, p1 k-projection spliced (front-loaded so
            # pair-2/3 k tiles are assembled well before use)
            for Q in range(NC_):
                if Q < 2:
                    for cc in (2 * Q, 2 * Q + 1):
                        fil.add(_qk_proj_pieces(nc, psp, tp, o12p,
                                                wqk[("k", 1)], cc, xtall,
                                                cos_sb, sin_sb,
                                                kpairs[2], kpairs[3], 1))
                _emit_pair_chunk(nc, psp, ptp, rp, 1, Q, qpairs[1],
                                 kpairs[1], vpall, yin[1][Q], tri2, fil)
            fil.drain()

            # pairs 2/3 in descending chunk order (so the kernel ends
            # on the smallest chunk), o_proj(Q) spliced into the
            # following chunks' attention streams
            fil.rate = 3
            for Q in range(NC_ - 1, -1, -1):
                _emit_pair_chunk(nc, psp, ptp, rp, 2, Q, qpairs[2],
                                 kpairs[2], vpall, yin[2][Q], tri2, fil)
                _emit_pair_chunk(nc, psp, ptp, rp, 3, Q, qpairs[3],
                                 kpairs[3], vpall, yin[3][Q], tri2, fil)
                fil.add(_oproj_pieces(nc, psp, ost, oct_sb,
                                      [yin[pr][Q] for pr in range(4)],
                                      OT, Q))
            fil.drain()

    nc.finalize()
    return nc


def _prep_inputs(x, q_proj, k_proj, v_proj, o_proj):
    pos = np.arange(S, dtype=np.float64)
    inv = THETA ** (-np.arange(0, DK, 2, dtype=np.float64) / DK)   # [32]
    ang = inv[:, None] * pos[None, :]                              # [32, S]
    cos_big = np.tile(np.cos(ang), (4, 1)).astype(bfloat16)
    sin_big = np.tile(np.sin(ang), (4, 1)).astype(bfloat16)

    def qk_layout(W, perm):
        # [2, 128, NK, 256]: per p-group, partition-major, contiguous
        wt = W[perm, :].T                     # [D, 512] cols = x1|x2 dims
        wt = wt.reshape(NK, 128, 512)         # [k, r, c]
        out = np.empty((2, 128, NK, 256), dtype=np.float32)
        for p in range(2):
            blk = np.concatenate(
                [wt[:, :, p * 128:(p + 1) * 128],
                 wt[:, :, 256 + p * 128:256 + (p + 1) * 128]], axis=2)
            out[p] = blk.transpose(1, 0, 2)
        return out.astype(bfloat16)

    in_maps = []
    for core in range(NCORES):
        b, g = core // 2, core % 2
        heads = [g * HPC + i for i in range(HPC)]
        rows_x1 = [h * DK + 2 * e for h in heads for e in range(32)]
        rows_x2 = [h * DK + 2 * e + 1 for h in heads for e in range(32)]
        perm = rows_x1 + rows_x2
        nat = [h * DK + d_ for h in heads for d_ in range(DK)]
        # XT: [128, NC_, NK, 512]; A[r, c, k, s'] = x[b, c*512+s', k*128+r]
        xt = np.ascontiguousarray(
            x[b].reshape(NC_, 512, NK, 128).transpose(3, 0, 2, 1))
        # WV: [128, NK, HD]
        wv = np.ascontiguousarray(
            v_proj[nat, :].T.reshape(NK, 128, HD).transpose(1, 0, 2))
        # OC: [128, 4, D]
        oc = np.ascontiguousarray(
            o_proj[:, nat].T.reshape(4, 128, D).transpose(1, 0, 2))
        in_maps.append({
            "XT": xt.astype(bfloat16),
            "WQ": qk_layout(q_proj, perm),
            "WK": qk_layout(k_proj, perm),
            "WV": wv.astype(bfloat16),
            "OC": oc.astype(bfloat16),
            "COS": cos_big,
            "SIN": sin_big,
        })
    return in_maps


def _run(in_maps, **kw):
    if "nc" not in _CACHE:
        _CACHE["nc"] = _build_nc()
    return run_bass_kernel_spmd(_CACHE["nc"], in_maps,
                                core_ids=list(range(NCORES)), **kw)


def kernel(x, q_proj, k_proj, v_proj, o_proj):
    x = np.asarray(x, dtype=np.float32)
    in_maps = _prep_inputs(x,
                           np.asarray(q_proj, dtype=np.float32),
                           np.asarray(k_proj, dtype=np.float32),
                           np.asarray(v_proj, dtype=np.float32),
                           np.asarray(o_proj, dtype=np.float32))
    res = _run(in_maps)
    B = x.shape[0]
    out = np.empty((B, S, D), dtype=np.float32)
    for b in range(B):
        ot = (res.results[2 * b]["OT"].astype(np.float32)
              + res.results[2 * b + 1]["OT"].astype(np.float32))
        out[b] = ot.T
    return out
